# revision 1
# baseline (speedup 1.0000x reference)
"""AdaptiveNoiseMask Trainium2 kernel, data-parallel over 8 NeuronCores.

out = x + where(rand_u < 0.3, noise_std * scale_row, 0)
scale_row = min(0.1 * (1 + max_softmax_prob(model_output)), 1.0)

max softmax prob per row = 1 / sum(exp(logits - max(logits))), so no full
softmax materialization is needed; the min() clamp never binds because the
confidence is in (0, 1] => scale in (0.1, 0.2].

Sharding: batch dim (4096) split 8 ways -> 512 rows per core, no
cross-core communication.
"""

import numpy as np

import concourse.bacc as bacc
import concourse.tile as tile
from concourse import mybir
from concourse.bass_utils import run_bass_kernel_spmd

N_CORES = 8
B, D, C = 4096, 4096, 1000
RB = B // N_CORES  # rows per core (512)
P = 128            # SBUF partitions
NT = RB // P       # row tiles per core (4)
# free-dim chunking of the main pass: full-width for the bulk, tapered
# chunks at the very end so the DVE+store tail after the last load is short
BULK_CHUNKS = [(0, 4096)]
TAIL_CHUNKS = [(0, 1792), (1792, 1024), (2816, 768), (3584, 512)]

NOISE_SCALE = 0.1
NOISE_RATIO = 0.3
ADAPTIVE_FACTOR = 1.0

_nc_cache = None


def build_bass():
    f32 = mybir.dt.float32
    nc = bacc.Bacc(
        "TRN2", target_bir_lowering=False, debug=False,
        # no collectives or per-core branching: partition-id is dead weight
        enable_partition_id=False,
    )

    # The const-AP MEMSETs bass emits in its preamble are dead weight here
    # (nothing consumes the const APs in this kernel) and they anchor the
    # profiler's first-useful-instruction window ~0.7us before the first
    # DMA trigger. Drop them before anything else references the block.
    entry = nc.main_func.blocks[0]
    for i in [i for i in entry.instructions if type(i).__name__ == "InstMemset"]:
        entry.instructions.remove(i)

    x_d = nc.dram_tensor("x", [RB, D], f32, kind="ExternalInput")
    mo_d = nc.dram_tensor("model_output", [RB, C], f32, kind="ExternalInput")
    u_d = nc.dram_tensor("rand_u", [RB, D], f32, kind="ExternalInput")
    ns_d = nc.dram_tensor("noise_std", [RB, D], f32, kind="ExternalInput")
    out_d = nc.dram_tensor("out", [RB, D], f32, kind="ExternalOutput")

    with tile.TileContext(nc) as tc:
        with (
            tc.tile_pool(name="mo", bufs=2) as mo_pool,
            tc.tile_pool(name="stats", bufs=8) as stats_pool,
            tc.tile_pool(name="scales", bufs=NT) as scale_pool,
            # bulk gets one buffer generation per chunk and the tail its own
            # small pool: no tile ever reuses a bulk buffer, so no tail load
            # trigger carries a recycle wait on a bulk SWDGE-store completion
            # (that wait gated the whole tail behind ~86us before)
            tc.tile_pool(name="big", bufs=3) as big_pool,
            tc.tile_pool(name="tailp", bufs=2) as tail_pool,
        ):
            # Phase 1: per-row noise scale from softmax confidence.
            scale_tiles = []
            for rt in range(NT):
                rows = slice(rt * P, (rt + 1) * P)
                mo_t = mo_pool.tile([P, C], f32, tag="mo")
                # ACT ring: keeps the sync ring free for the big streaming loads
                nc.scalar.dma_start(out=mo_t[:], in_=mo_d.ap()[rows, :])
                negmax = stats_pool.tile([P, 1], f32, tag="negmax")
                nc.vector.reduce_max(
                    out=negmax[:], in_=mo_t[:], axis=mybir.AxisListType.X,
                    negate=True,
                )
                sumexp = stats_pool.tile([P, 1], f32, tag="sumexp")
                nc.scalar.activation(
                    out=mo_t[:], in_=mo_t[:],
                    func=mybir.ActivationFunctionType.Exp,
                    bias=negmax[:], scale=1.0, accum_out=sumexp[:],
                )
                conf = stats_pool.tile([P, 1], f32, tag="conf")
                nc.vector.reciprocal(out=conf[:], in_=sumexp[:])
                sc = scale_pool.tile([P, 1], f32, tag=f"scale{rt}")
                # scale = conf * (NOISE_SCALE*ADAPTIVE_FACTOR) + NOISE_SCALE
                nc.vector.tensor_scalar(
                    out=sc[:], in0=conf[:],
                    scalar1=NOISE_SCALE * ADAPTIVE_FACTOR, scalar2=NOISE_SCALE,
                    op0=mybir.AluOpType.mult, op1=mybir.AluOpType.add,
                )
                scale_tiles.append(sc)

            # Phase 2: streaming masked-noise add.
            # Work order interleaves the last row-tile's tapered pieces into
            # the bulk stream (c0, p0, c1, p1, c2, p2, p3): pieces p0-p2's
            # stores complete while loads still stream, so after the very
            # last load (p3's 512-col x, 256KB) only one short
            # stt2+store+receipt chain is exposed.
            work = []
            for k, (c0, cw) in enumerate(TAIL_CHUNKS):
                if k < NT - 1:
                    work.append((k, BULK_CHUNKS[0]))
                work.append((NT - 1, (c0, cw)))
            for rt, (c0, cw) in work:
                rows = slice(rt * P, (rt + 1) * P)
                pool = tail_pool if rt == NT - 1 else big_pool
                cols = slice(c0, c0 + cw)
                xt = pool.tile([P, cw], f32, tag="x")
                ut = pool.tile([P, cw], f32, tag="u")
                nt_ = pool.tile([P, cw], f32, tag="n")
                # u and n first: stt1 needs them; x is only needed by
                # stt2 so its load overlaps stt1
                nc.sync.dma_start(out=ut[:], in_=u_d.ap()[rows, cols])
                # n on the ACT ring balances the two HWDGE rings
                # (sync: u+x = 16.8MB, scalar: n+mo = 10.4MB)
                nc.scalar.dma_start(out=nt_[:], in_=ns_d.ap()[rows, cols])
                nc.sync.dma_start(out=xt[:], in_=x_d.ap()[rows, cols])
                # ut = (u < 0.3) * noise
                nc.vector.scalar_tensor_tensor(
                    out=ut[:], in0=ut[:], scalar=NOISE_RATIO, in1=nt_[:],
                    op0=mybir.AluOpType.is_lt, op1=mybir.AluOpType.mult,
                )
                # xt = ut * scale_row + x
                nc.vector.scalar_tensor_tensor(
                    out=xt[:], in0=ut[:], scalar=scale_tiles[rt][:],
                    in1=xt[:],
                    op0=mybir.AluOpType.mult, op1=mybir.AluOpType.add,
                )
                # bulk stores go out the SWDGE path: keeps both HWDGE
                # rings exclusively feeding loads (measured faster). The
                # small tapered tail-piece stores use the ACT ring; all but
                # the last complete while loads still stream
                if rt == NT - 1:
                    nc.scalar.dma_start(out=out_d.ap()[rows, cols],
                                        in_=xt[:])
                else:
                    nc.gpsimd.dma_start(out=out_d.ap()[rows, cols],
                                        in_=xt[:])

    nc.compile()
    return nc


def _get_nc():
    global _nc_cache
    if _nc_cache is None:
        _nc_cache = build_bass()
    return _nc_cache


def kernel(x, model_output, rand_u, noise_std, **run_kwargs):
    nc = _get_nc()
    x = np.ascontiguousarray(x, dtype=np.float32)
    model_output = np.ascontiguousarray(model_output, dtype=np.float32)
    rand_u = np.ascontiguousarray(rand_u, dtype=np.float32)
    noise_std = np.ascontiguousarray(noise_std, dtype=np.float32)

    in_maps = []
    for i in range(N_CORES):
        rows = slice(i * RB, (i + 1) * RB)
        in_maps.append({
            "x": x[rows],
            "model_output": model_output[rows],
            "rand_u": rand_u[rows],
            "noise_std": noise_std[rows],
        })

    res = run_bass_kernel_spmd(nc, in_maps, core_ids=list(range(N_CORES)),
                               **run_kwargs)
    out = np.concatenate([res.results[i]["out"] for i in range(N_CORES)],
                         axis=0)
    kernel.last_result = res
    return out

